# revision 15
# baseline (speedup 1.0000x reference)
"""Trainium2 Bass kernel for nn_CaevlFT_39367670235990 (retrieval_knn VICReg-style loss).

Strategy (2 SPMD launches over 8 cores, no collectives):
  Launch 1 (batch-sharded, 32 samples/core): per-sample KNN matching
    (feature-space + location-space), argmin one-hots, rank-based selection,
    PE-transposes of the map tiles, and one-hot-matmul row gathers.
    Outputs transposed map rows + gathered rows per sample.
  Host: reshard rows from batch-major to position(m)-major pairs (pure memcpy).
  Launch 2 (m-sharded): cross-batch statistics per position m:
    invariance partial sums, per-(m,c) variance stats, and covariance
    Frobenius norms via the Gram trick ||X^T X||_F = ||X X^T||_F with
    G = xc xc^T (256x256), contraction over C on the PE at f32r rate.
  Host: tiny scalar epilogue combining per-core partial sums.

All shapes hardcoded for B=256, C=512, HW=49, D=8192, 8 cores.
"""

import os
import sys
import numpy as np

for p in ("/opt/trn_rl_repo", "/opt/pypackages"):
    if p not in sys.path:
        sys.path.insert(0, p)

import concourse.bass as bass
import concourse.bacc as bacc
import concourse.tile as tile
from concourse import mybir
from concourse.bass_utils import run_bass_kernel_spmd

F32 = mybir.dt.float32
F32R = mybir.dt.float32r
AX = mybir.AxisListType
OP = mybir.AluOpType
AF = mybir.ActivationFunctionType

NCORES = 8
B = 256
BL = B // NCORES          # 32 samples per core in launch 1
C = 512
HW = 49
D = 8192
BIG = 1024.0  # > 49 and small enough that j-BIG is exact in f32
EPS = 1e-5

# per-core pair count in launch 2: 122 real pairs (49+49+20+4) padded to 128
NPAIR = 16
GCHUNK = D // NCORES // 128   # 8 chunks of (128,256) per global tensor per core


# ----------------------------------------------------------------------------
# constants shared with the device
# ----------------------------------------------------------------------------
def _grid():
    c = (np.arange(7, dtype=np.float32) + 0.5) * (224.0 / 7.0)
    gx = np.repeat(c[:, None], 7, axis=1)
    gy = np.repeat(c[None, :], 7, axis=0)
    return np.stack([gx, gy], axis=-1).reshape(49, 2)  # (49,2)


def _phase1_consts(bl=BL):
    g = _grid()
    lt = np.zeros((49, 49), np.float32)  # lt[i, ip] = 1 if ip < i
    for i in range(49):
        lt[i, :i] = 1.0
    iota49 = np.arange(49, dtype=np.float32)
    return {
        "ones49": np.ones((1, 49), np.float32),
        "ones128": np.ones((128, 1), np.float32),
        "ident": np.eye(128, dtype=np.float32),
        "gridT": np.ascontiguousarray(g.T),                      # (2,49)
        "g2m05": (-0.5 * (g * g).sum(1))[None, :].astype(np.float32),  # (1,49)
        "g2col": ((g * g).sum(1))[:, None].astype(np.float32),   # (49,1)
        "iota49c": iota49[:, None].copy(),
        "mhalf2": np.full((2, 49), -0.5, np.float32),                       # (49,1)
        "iotaJbc": np.tile(iota49[None, :], (49, 1)),            # (49,49)
        "iota20bc": np.tile(np.arange(1, 21, dtype=np.float32)[None, :], (49, 1)),
        "iota4bc": np.tile(np.arange(1, 5, dtype=np.float32)[None, :], (49, 1)),
        "lt_bl": np.tile(lt.reshape(1, 2401), (bl, 1)),          # (bl,2401)
    }


# ----------------------------------------------------------------------------
# Launch 1: per-sample matching + gathers (batch-sharded)
# ----------------------------------------------------------------------------
def build_phase1(bl=BL):
    nc = bacc.Bacc("TRN2", target_bir_lowering=False, debug=False,
                   enable_asserts=False, num_devices=NCORES)

    m1f = nc.dram_tensor("m1f", [bl, 128, 196], F32, kind="ExternalInput").ap()
    m2f = nc.dram_tensor("m2f", [bl, 128, 196], F32, kind="ExternalInput").ap()
    locT = nc.dram_tensor("locT", [bl, 2, 49], F32, kind="ExternalInput").ap()
    locN = nc.dram_tensor("locN", [bl, 49, 2], F32, kind="ExternalInput").ap()
    cst = {k: nc.dram_tensor(k, list(v.shape), F32, kind="ExternalInput").ap()
           for k, v in _phase1_consts(bl).items()}

    o_m1T = nc.dram_tensor("o_m1T", [bl, 49, 512], F32, kind="ExternalOutput").ap()
    o_m2T = nc.dram_tensor("o_m2T", [bl, 49, 512], F32, kind="ExternalOutput").ap()
    o_sel1 = nc.dram_tensor("o_sel1", [bl, 73, 512], F32, kind="ExternalOutput").ap()
    o_sel2 = nc.dram_tensor("o_sel2", [bl, 73, 512], F32, kind="ExternalOutput").ap()

    with tile.TileContext(nc) as tc:
        with (
            tc.tile_pool(name="big", bufs=1) as big,
            tc.tile_pool(name="cpool", bufs=1) as cpool,
            tc.tile_pool(name="work", bufs=3) as work,
            tc.tile_pool(name="outp", bufs=3) as outp,
            tc.tile_pool(name="pd", bufs=4, space=bass.MemorySpace.PSUM) as pd,
            tc.tile_pool(name="pt", bufs=2, space=bass.MemorySpace.PSUM) as pt,
            tc.tile_pool(name="ps", bufs=2, space=bass.MemorySpace.PSUM) as ps,
        ):
            # ---- load constants to SBUF
            cs = {}
            for k, v in _phase1_consts(bl).items():
                t = cpool.tile(list(v.shape), F32, tag=f"c_{k}", name=f"ct_{k}")
                nc.sync.dma_start(t[:], cst[k])
                cs[k] = t

            # ---- load all local samples (c-grouped native layout)
            T1 = big.tile([128, bl, 196], F32, tag="T1")
            T2 = big.tile([128, bl, 196], F32, tag="T2")
            nc.sync.dma_start(T1[:], m1f.rearrange("s p f -> p s f"))
            nc.sync.dma_start(T2[:], m2f.rearrange("s p f -> p s f"))

            # ---- row norms: s_row = -0.5 * sum_c x_c^2  (per sample, per hw pos)
            srow = []
            for T, tag in ((T1, "s1"), (T2, "s2")):
                sq = work.tile([128, bl, 196], F32, tag="sq", bufs=1)
                nc.vector.tensor_tensor(sq[:], T[:], T[:], OP.mult)
                f0 = work.tile([128, bl, 49], F32, tag="fold0", bufs=1)
                nc.vector.tensor_tensor(f0[:], sq[:, :, 0:49], sq[:, :, 49:98], OP.add)
                f1 = work.tile([128, bl, 49], F32, tag="fold1", bufs=1)
                nc.vector.tensor_tensor(f1[:], sq[:, :, 98:147], sq[:, :, 147:196], OP.add)
                nc.vector.tensor_tensor(f0[:], f0[:], f1[:], OP.add)
                sr = big.tile([1, bl * 49], F32, tag=f"srow_{tag}")
                f0f = f0[:].rearrange("p s f -> p (s f)")
                for off in range(0, bl * 49, 512):
                    w = min(512, bl * 49 - off)
                    prow = ps.tile([1, w], F32, tag="psmall", name=f"prow_{tag}_{off}")
                    nc.tensor.matmul(prow[:], cs["ones128"][:], f0f[:, off:off + w],
                                     start=True, stop=True)
                    nc.vector.tensor_scalar(sr[:, off:off + w], prow[:], -0.5, None, OP.mult)
                srow.append(sr)
            s1row, s2row = srow

            # ---- per-sample distance matrices (max-form: dot - 0.5*|cand|^2)
            Dall = big.tile([49, bl, 49], F32, tag="Dall")
            D2all = big.tile([49, bl, 49], F32, tag="D2all")
            DLall = big.tile([49, bl, 49], F32, tag="DLall")
            DLTall = big.tile([49, bl, 49], F32, tag="DLTall")
            l2all = big.tile([49, bl], F32, tag="l2all")

            for s in range(bl):
                # feature-space: D[i,j] = m1_i . m2_j - 0.5|m2_j|^2   (argmax_j)
                Dp = pd.tile([49, 49], F32, tag="dmat", name=f"Dp_{s}")
                for q in range(4):
                    nc.tensor.matmul(Dp[:], T1[:, s, q * 49:(q + 1) * 49],
                                     T2[:, s, q * 49:(q + 1) * 49],
                                     start=(q == 0), stop=False)
                nc.tensor.matmul(Dp[:], cs["ones49"][:],
                                 s2row[:, s * 49:(s + 1) * 49], start=False, stop=True)
                nc.vector.tensor_copy(Dall[:, s, :], Dp[:])

                D2p = pd.tile([49, 49], F32, tag="dmat", name=f"D2p_{s}")
                for q in range(4):
                    nc.tensor.matmul(D2p[:], T2[:, s, q * 49:(q + 1) * 49],
                                     T1[:, s, q * 49:(q + 1) * 49],
                                     start=(q == 0), stop=False)
                nc.tensor.matmul(D2p[:], cs["ones49"][:],
                                 s1row[:, s * 49:(s + 1) * 49], start=False, stop=True)
                nc.vector.tensor_copy(D2all[:, s, :], D2p[:])

                # location-space
                lT = work.tile([2, 49], F32, tag="lT")
                nc.sync.dma_start(lT[:], locT[s])
                lN = work.tile([49, 2], F32, tag="lN")
                nc.sync.dma_start(lN[:], locN[s])
                lsqN = work.tile([49, 2], F32, tag="lsqN")
                nc.vector.tensor_tensor(lsqN[:], lN[:], lN[:], OP.mult)
                nc.vector.tensor_reduce(l2all[:, s:s + 1], lsqN[:], AX.X, OP.add)
                lsqT = work.tile([2, 49], F32, tag="lsqT")
                nc.vector.tensor_tensor(lsqT[:], lT[:], lT[:], OP.mult)

                DLp = pd.tile([49, 49], F32, tag="dmat", name=f"DLp_{s}")
                nc.tensor.matmul(DLp[:], cs["gridT"][:], lT[:], start=True, stop=False)
                nc.tensor.matmul(DLp[:], cs["mhalf2"][:], lsqT[:], start=False, stop=True)
                nc.vector.tensor_copy(DLall[:, s, :], DLp[:])

                DLTp = pd.tile([49, 49], F32, tag="dmat", name=f"DLTp_{s}")
                nc.tensor.matmul(DLTp[:], lT[:], cs["gridT"][:], start=True, stop=False)
                nc.tensor.matmul(DLTp[:], cs["ones49"][:], cs["g2m05"][:],
                                 start=False, stop=True)
                nc.vector.tensor_copy(DLTall[:, s, :], DLTp[:])

            # ---- batched argmax + first-occurrence index
            def argmax_idx(Mall, tagp):
                mx = big.tile([49, bl], F32, tag=f"mx_{tagp}")
                nc.vector.tensor_reduce(mx[:], Mall[:], AX.X, OP.max)
                eq = work.tile([49, bl, 49], F32, tag="eq", bufs=1)
                nc.vector.tensor_tensor(eq[:], Mall[:],
                                        mx[:, :, None].broadcast_to((49, bl, 49)),
                                        OP.is_equal)
                cc = eq
                nc.vector.tensor_scalar(cc[:], eq[:], -BIG, None, OP.mult)
                nc.vector.tensor_tensor(
                    cc[:], cc[:],
                    cs["iotaJbc"][:, None, :].broadcast_to((49, bl, 49)),
                    OP.add)
                idx = big.tile([49, bl], F32, tag=f"idx_{tagp}")
                nc.vector.tensor_reduce(idx[:], cc[:], AX.X, OP.min)
                nc.vector.tensor_scalar(idx[:], idx[:], BIG, None, OP.add)
                return mx, idx

            _, idx1 = argmax_idx(Dall, "d1")      # (49m, bl) : j1 into m2 rows
            _, idx2 = argmax_idx(D2all, "d2")     # j2 into m1 rows
            mxL, idxL = argmax_idx(DLall, "dl")   # per grid-pos i: nearest loc idx
            mxL2, idxL2 = argmax_idx(DLTall, "dl2")  # per loc-pos j: nearest grid idx

            # nn values for ranking (true squared distances)
            nnL = big.tile([49, bl], F32, tag="nnL")
            nc.vector.tensor_scalar(nnL[:], mxL[:], -2.0, cs["g2col"][:], OP.mult, OP.add)
            nnL2 = big.tile([49, bl], F32, tag="nnL2")
            nc.vector.tensor_scalar(nnL2[:], mxL2[:], -2.0, None, OP.mult)
            nc.vector.tensor_tensor(nnL2[:], nnL2[:], l2all[:], OP.add)

            # ---- transpose helper: (p,f) -> (f,p)
            def tTr(src, pdim, fdim, tagp):
                pp = ps.tile([fdim, pdim], F32, tag="psmall", name=f"tTrp_{tagp}")
                nc.tensor.transpose(pp[:], src[:], cs["ident"][0:pdim, 0:pdim])
                t = big.tile([fdim, pdim], F32, tag=f"tTr_{tagp}",
                             name=f"tTr_{tagp}")
                nc.vector.tensor_copy(t[:], pp[:])
                return t

            nnLT = tTr(nnL, 49, bl, "nnL")
            nnL2T = tTr(nnL2, 49, bl, "nnL2")

            # ---- ranking + selection one-hots (branch 2)
            def sel_onehot(nnT, k, iota_k, tagp):
                # rank[b,i] = #{i': nn[i']<nn[i]} + #{i'<i: nn[i']==nn[i]}
                # out layout [b, i, i']: in0 = nn[b,i'] (inner), in1 = nn[b,i]
                in0 = nnT[:, None, :].broadcast_to((bl, 49, 49))       # [b,i,i'] = nn[b,i']
                in1 = nnT[:, :, None].broadcast_to((bl, 49, 49))       # [b,i,i'] = nn[b,i]
                cl = work.tile([bl, 49, 49], F32, tag="cl", name=f"cl_{tagp}", bufs=1)
                nc.vector.tensor_tensor(cl[:], in0, in1, OP.is_lt)
                ce = work.tile([bl, 49, 49], F32, tag="ce", name=f"ce_{tagp}", bufs=1)
                nc.vector.tensor_tensor(ce[:], in0, in1, OP.is_equal)
                ltb = cs["lt_bl"][:].rearrange("b (i j) -> b i j", i=49)
                nc.vector.tensor_tensor(ce[:], ce[:], ltb, OP.mult)
                nc.vector.tensor_tensor(cl[:], cl[:], ce[:], OP.add)
                rank = work.tile([bl, 49], F32, tag="rank", name=f"rank_{tagp}")
                nc.vector.tensor_reduce(rank[:], cl[:], AX.X, OP.add)
                mask = big.tile([bl, 49], F32, tag=f"mask_{tagp}",
                                name=f"mask_{tagp}")
                nc.vector.tensor_scalar(mask[:], rank[:], k - 0.5, None, OP.is_lt)
                # inclusive prefix sum along free dim (log steps, ping-pong)
                ca = work.tile([bl, 49], F32, tag="csA", name=f"csA_{tagp}")
                cb = work.tile([bl, 49], F32, tag="csB", name=f"csB_{tagp}")
                nc.vector.tensor_copy(ca[:], mask[:])
                cur, nxt = ca, cb
                for sh in (1, 2, 4, 8, 16, 32):
                    if sh >= 49:
                        break
                    nc.vector.tensor_copy(nxt[:], cur[:])
                    nc.vector.tensor_tensor(nxt[:, sh:49], cur[:, sh:49],
                                            cur[:, 0:49 - sh], OP.add)
                    cur, nxt = nxt, cur
                maskT = tTr(mask, bl, 49, f"maskT_{tagp}")   # (49, bl)
                csumT = tTr(cur, bl, 49, f"csumT_{tagp}")    # (49, bl)
                # E[i,(b,s)] = mask[b,i] * [csum[b,i] == s+1]
                E = big.tile([49, bl, k], F32, tag=f"E_{tagp}", name=f"E_{tagp}")
                nc.vector.tensor_tensor(
                    E[:], csumT[:, :, None].broadcast_to((49, bl, k)),
                    iota_k[:, None, :].broadcast_to((49, bl, k)),
                    OP.is_equal)
                nc.vector.tensor_tensor(
                    E[:], E[:], maskT[:, :, None].broadcast_to((49, bl, k)), OP.mult)
                return E

            E1b2 = sel_onehot(nnLT, 20, cs["iota20bc"], "s20")   # sel over grid pos
            E2b2 = sel_onehot(nnL2T, 4, cs["iota4bc"], "s4")     # sel over loc pos

            # ---- per-sample map transposes, one-hot builds, row gathers
            for s in range(bl):
                mTs = []
                for T, oT in ((T1, o_m1T), (T2, o_m2T)):
                    mp = pt.tile([49, 512], F32, tag="pbig", name=f"mTp_{s}_{0 if oT is o_m1T else 1}")
                    for q in range(4):
                        nc.tensor.transpose(mp[:, q * 128:(q + 1) * 128],
                                            T[:, s, q * 49:(q + 1) * 49],
                                            cs["ident"][:])
                    mt = outp.tile([49, 512], F32, tag="mTs")
                    nc.vector.tensor_copy(mt[:].bitcast(F32R), mp[:])
                    nc.sync.dma_start(oT[s], mt[:])
                    mTs.append(mt)
                m1Ts, m2Ts = mTs

                # E matrices for gathers from m2 (n1 | n1b2 | f2b2)
                E2f = work.tile([49, 73], F32, tag="E2f")
                tN1 = work.tile([49, 49], F32, tag="tN1", name=f"tN1_{s}")
                nc.vector.tensor_tensor(tN1[:],
                                        idx1[:, s:s + 1].broadcast_to((49, 49)),
                                        cs["iotaJbc"][:], OP.is_equal)
                pn1 = ps.tile([49, 49], F32, tag="psmall", name=f"pn1_{s}")
                nc.tensor.transpose(pn1[:], tN1[:], cs["ident"][0:49, 0:49])
                nc.vector.tensor_copy(E2f[:, 0:49].bitcast(F32R), pn1[:])
                tmpE = work.tile([49, 49], F32, tag="tmpE")
                nc.vector.tensor_tensor(
                    tmpE[:],
                    idxL[:, s:s + 1].broadcast_to((49, 49)),
                    cs["iotaJbc"][:], OP.is_equal)                     # [i,j] = [idxL(i)==j]
                cmp1 = ps.tile([49, 20], F32, tag="psmall", name=f"cmp1_{s}")
                nc.tensor.matmul(cmp1[:], tmpE[:], E1b2[:, s, :], start=True, stop=True)
                nc.vector.tensor_copy(E2f[:, 49:69].bitcast(F32R), cmp1[:])
                nc.vector.tensor_copy(E2f[:, 69:73].bitcast(F32R), E2b2[:, s, :])

                # E matrices for gathers from m1 (n2 | f1b2 | n2b2)
                E1f = work.tile([49, 73], F32, tag="E1f")
                tN2 = work.tile([49, 49], F32, tag="tN2", name=f"tN2_{s}")
                nc.vector.tensor_tensor(tN2[:],
                                        idx2[:, s:s + 1].broadcast_to((49, 49)),
                                        cs["iotaJbc"][:], OP.is_equal)
                pn2 = ps.tile([49, 49], F32, tag="psmall", name=f"pn2_{s}")
                nc.tensor.transpose(pn2[:], tN2[:], cs["ident"][0:49, 0:49])
                nc.vector.tensor_copy(E1f[:, 0:49].bitcast(F32R), pn2[:])
                tmpE2 = work.tile([49, 49], F32, tag="tmpE2")
                nc.vector.tensor_tensor(
                    tmpE2[:],
                    idxL2[:, s:s + 1].broadcast_to((49, 49)),
                    cs["iotaJbc"][:], OP.is_equal)                     # [j,i] = [idxL2(j)==i]
                cmp2 = ps.tile([49, 4], F32, tag="psmall", name=f"cmp2_{s}")
                nc.tensor.matmul(cmp2[:], tmpE2[:], E2b2[:, s, :], start=True, stop=True)
                nc.vector.tensor_copy(E1f[:, 49:69].bitcast(F32R), E1b2[:, s, :])
                nc.vector.tensor_copy(E1f[:, 69:73].bitcast(F32R), cmp2[:])

                # gather rows
                P2 = pt.tile([73, 512], F32, tag="pbig", name=f"P2_{s}")
                nc.tensor.matmul(P2[:], E2f[:].bitcast(F32R), m2Ts[:].bitcast(F32R),
                                 start=True, stop=True)
                g2 = outp.tile([73, 512], F32, tag="g2")
                nc.vector.tensor_copy(g2[:], P2[:])
                nc.sync.dma_start(o_sel2[s], g2[:])

                P1 = pt.tile([73, 512], F32, tag="pbig", name=f"P1_{s}")
                nc.tensor.matmul(P1[:], E1f[:].bitcast(F32R), m1Ts[:].bitcast(F32R),
                                 start=True, stop=True)
                g1 = outp.tile([73, 512], F32, tag="g1")
                nc.vector.tensor_copy(g1[:], P1[:])
                nc.sync.dma_start(o_sel1[s], g1[:])

    nc.compile()
    return nc


# ----------------------------------------------------------------------------
# Launch 2: cross-batch statistics (m-sharded)
# ----------------------------------------------------------------------------
def build_phase2(npair=NPAIR, gchunk=GCHUNK):
    nc = bacc.Bacc("TRN2", target_bir_lowering=False, debug=False,
                   enable_asserts=False, num_devices=NCORES)

    pairs = nc.dram_tensor("pairs", [npair, 2, 4, 128, 256], F32,
                           kind="ExternalInput").ap()
    gp = nc.dram_tensor("gp", [2, gchunk, 128, 256], F32, kind="ExternalInput").ap()
    ones128 = nc.dram_tensor("ones128", [128, 1], F32, kind="ExternalInput").ap()

    inv_o = nc.dram_tensor("inv_o", [npair, 256], F32, kind="ExternalOutput").ap()
    r_o = nc.dram_tensor("r_o", [128, npair * 8], F32, kind="ExternalOutput").ap()
    s_o = nc.dram_tensor("s_o", [128, npair * 8], F32, kind="ExternalOutput").ap()
    g_o = nc.dram_tensor("g_o", [128, npair * 4], F32, kind="ExternalOutput").ap()
    gm_o = nc.dram_tensor("gm_o", [4, 128, 256], F32, kind="ExternalOutput").ap()
    ginv_o = nc.dram_tensor("ginv_o", [1, 256], F32, kind="ExternalOutput").ap()
    gr_o = nc.dram_tensor("gr_o", [128, 2 * gchunk], F32, kind="ExternalOutput").ap()
    gs_o = nc.dram_tensor("gs_o", [128, 2 * gchunk], F32, kind="ExternalOutput").ap()

    with tile.TileContext(nc) as tc:
        with (
            tc.tile_pool(name="cpool", bufs=1) as cpool,
            tc.tile_pool(name="stage", bufs=1) as stage,
            tc.tile_pool(name="work", bufs=3) as work,
            tc.tile_pool(name="pg", bufs=1, space=bass.MemorySpace.PSUM) as pg,
            tc.tile_pool(name="pi", bufs=2, space=bass.MemorySpace.PSUM) as pi,
        ):
            onesT = cpool.tile([128, 1], F32, tag="ones")
            ones_raw = cpool.tile([128, 1], F32, tag="ones_raw")
            nc.gpsimd.memset(ones_raw[:], 1.0)
            nc.vector.tensor_copy(onesT[:].bitcast(F32R), ones_raw[:])
            _ = ones128  # kept in the I/O signature; value unused on device
            epsv = cpool.tile([128, 1], F32, tag="epsv")
            nc.gpsimd.memset(epsv[:], EPS)

            rS = stage.tile([128, npair * 8], F32, tag="rS")
            sS = stage.tile([128, npair * 8], F32, tag="sS")
            gS = stage.tile([128, npair * 4], F32, tag="gS")
            grS = stage.tile([128, 2 * gchunk], F32, tag="grS")
            gsS = stage.tile([128, 2 * gchunk], F32, tag="gsS")

            def stats_side(X, nchunk, sObuf, rObuf, scol, gpsum_list):
                """X: sbuf (128, nchunk, 256) raw. Returns centered xc tile.
                Writes raw sumsq cols into sObuf[:, scol:scol+nchunk], relu(1-std)
                cols into rObuf, and accumulates G into gpsum_list (2 psum tiles)."""
                sums = work.tile([128, nchunk], F32, tag="sums")
                nc.vector.tensor_reduce(sums[:], X[:], AX.X, OP.add)
                mu = work.tile([128, nchunk], F32, tag="mu")
                nc.vector.tensor_scalar(mu[:], sums[:], 1.0 / 256.0, None, OP.mult)
                xc = work.tile([128, nchunk, 256], F32, tag="xc")
                for k in range(nchunk):
                    nc.vector.tensor_scalar(xc[:, k, :].bitcast(F32R), X[:, k, :],
                                            mu[:, k:k + 1], None, OP.subtract)
                # sumsq via ACT square-accumulate
                sqscr = work.tile([128, 256], F32, tag="sqscr")
                for k in range(nchunk):
                    nc.scalar.activation(sqscr[:], xc[:, k, :], AF.Square,
                                         accum_out=sObuf[:, scol + k:scol + k + 1])
                # relu(1 - sqrt(var+eps))
                var = work.tile([128, nchunk], F32, tag="var")
                nc.vector.tensor_scalar(var[:], sObuf[:, scol:scol + nchunk],
                                        1.0 / 255.0, None, OP.mult)
                stdv = work.tile([128, nchunk], F32, tag="stdv")
                nc.scalar.activation(stdv[:], var[:], AF.Sqrt, bias=epsv[:])
                nc.vector.tensor_scalar(stdv[:], stdv[:], -1.0, 1.0, OP.mult, OP.add)
                nc.vector.tensor_scalar(rObuf[:, scol:scol + nchunk], stdv[:],
                                        0.0, None, OP.max)
                # G accumulation (2 partition-halves of the 256-wide Gram)
                for m in range(2):
                    for k in range(nchunk):
                        nc.tensor.matmul(
                            gpsum_list[m][:],
                            xc[:, k, m * 128:(m + 1) * 128].bitcast(F32R),
                            xc[:, k, :].bitcast(F32R),
                            start=(k == 0), stop=(k == nchunk - 1))
                return xc

            for t in range(npair):
                Xs = []
                for side in range(2):
                    X = work.tile([128, 4, 256], F32, tag=f"X{side}")
                    nc.sync.dma_start(X[:], pairs[t, side].rearrange("k p n -> p k n"))
                    Xs.append(X)
                Gp = [pg.tile([128, 256], F32, tag=f"G{m}", name=f"Gp{m}_{t}") for m in range(2)]
                Gq = [pg.tile([128, 256], F32, tag=f"G{2+m}", name=f"Gq{m}_{t}") for m in range(2)]
                xcx = stats_side(Xs[0], 4, sS, rS, t * 8, Gp)
                xcy = stats_side(Xs[1], 4, sS, rS, t * 8 + 4, Gq)
                # sum G^2 (via ACT square-accum over psum)
                sqg = work.tile([128, 256], F32, tag="sqg")
                for m in range(2):
                    nc.scalar.activation(sqg[:], Gp[m][:], AF.Square,
                                         accum_out=gS[:, t * 4 + m:t * 4 + m + 1])
                    nc.scalar.activation(sqg[:], Gq[m][:], AF.Square,
                                         accum_out=gS[:, t * 4 + 2 + m:t * 4 + 3 + m])
                # invariance: sum_c (x-y)^2 per column b (raw values)
                ip = pi.tile([1, 256], F32, tag="ip")
                for k in range(4):
                    df = work.tile([128, 256], F32, tag="df")
                    nc.vector.tensor_tensor(df[:].bitcast(F32R), Xs[0][:, k, :],
                                            Xs[1][:, k, :], OP.subtract)
                    nc.vector.tensor_tensor(df[:].bitcast(F32R), df[:], df[:],
                                            OP.mult)
                    nc.tensor.matmul(ip[:], onesT[:].bitcast(F32R),
                                     df[:].bitcast(F32R),
                                     start=(k == 0), stop=(k == 3))
                iv = work.tile([1, 256], F32, tag="iv")
                nc.vector.tensor_copy(iv[:], ip[:])
                nc.sync.dma_start(inv_o[t], iv[:])

            # ---- global embedding block (1024 cols of D=8192 per core)
            Xg = []
            for side in range(2):
                X = stage.tile([128, gchunk, 256], F32, tag=f"Xg{side}")
                nc.sync.dma_start(X[:], gp[side].rearrange("k p n -> p k n"))
                Xg.append(X)
            Gg = [[pg.tile([128, 256], F32, tag=f"G{side*2+m}", name=f"Gg{side}{m}") for m in range(2)]
                  for side in range(2)]
            for side in range(2):
                stats_side(Xg[side], gchunk, gsS, grS, side * gchunk, Gg[side])
                for m in range(2):
                    gm = work.tile([128, 256], F32, tag="gm")
                    nc.vector.tensor_copy(gm[:], Gg[side][m][:])
                    nc.sync.dma_start(gm_o[side * 2 + m], gm[:])
            gip = pi.tile([1, 256], F32, tag="gip")
            for k in range(gchunk):
                df = work.tile([128, 256], F32, tag="gdf")
                nc.vector.tensor_tensor(df[:].bitcast(F32R), Xg[0][:, k, :],
                                        Xg[1][:, k, :], OP.subtract)
                nc.vector.tensor_tensor(df[:].bitcast(F32R), df[:], df[:], OP.mult)
                nc.tensor.matmul(gip[:], onesT[:].bitcast(F32R), df[:].bitcast(F32R),
                                 start=(k == 0), stop=(k == gchunk - 1))
            giv = work.tile([1, 256], F32, tag="giv")
            nc.vector.tensor_copy(giv[:], gip[:])
            nc.sync.dma_start(ginv_o[0:1, :], giv[:])

            nc.sync.dma_start(r_o, rS[:])
            nc.sync.dma_start(s_o, sS[:])
            nc.sync.dma_start(g_o, gS[:])
            nc.sync.dma_start(gr_o, grS[:])
            nc.sync.dma_start(gs_o, gsS[:])

    nc.compile()
    return nc


# ----------------------------------------------------------------------------
# host orchestration
# ----------------------------------------------------------------------------
_NC1 = None
_NC2 = None


def _get_ncs():
    global _NC1, _NC2
    if _NC1 is None:
        _NC1 = build_phase1()
    if _NC2 is None:
        _NC2 = build_phase2()
    return _NC1, _NC2


def kernel(maps_1, maps_2, projected_x, projected_y, locations, _return_time=False):
    nc1, nc2 = _get_ncs()
    m1 = np.ascontiguousarray(maps_1.reshape(B, C, HW), np.float32)
    m2 = np.ascontiguousarray(maps_2.reshape(B, C, HW), np.float32)
    loc = np.ascontiguousarray(locations, np.float32)
    consts = _phase1_consts()

    in_maps1 = []
    for k in range(NCORES):
        sl = slice(k * BL, (k + 1) * BL)
        im = {
            "m1f": m1[sl].reshape(BL, 128, 196),
            "m2f": m2[sl].reshape(BL, 128, 196),
            "locT": np.ascontiguousarray(loc[sl].transpose(0, 2, 1)),
            "locN": loc[sl],
        }
        im.update(consts)
        in_maps1.append(im)

    trace = bool(os.environ.get("KBENCH_TRACE"))
    r1 = run_bass_kernel_spmd(nc1, in_maps1, core_ids=list(range(NCORES)),
                              trace=trace)
    t1 = r1.exec_time_ns

    m1T = np.concatenate([r["o_m1T"] for r in r1.results], 0)    # (256,49,512)
    m2T = np.concatenate([r["o_m2T"] for r in r1.results], 0)
    sel1 = np.concatenate([r["o_sel1"] for r in r1.results], 0)  # (256,73,512)
    sel2 = np.concatenate([r["o_sel2"] for r in r1.results], 0)

    groups = {
        "m1": m1T, "m2": m2T,
        "n1": sel2[:, 0:49], "n2": sel1[:, 0:49],
        "f1b2": sel1[:, 49:69], "n1b2": sel2[:, 49:69],
        "f2b2": sel2[:, 69:73], "n2b2": sel1[:, 69:73],
    }
    # pair list: (x_group, y_group, m, loss_tag)
    plist = ([("m1", "n1", m, "L1a") for m in range(49)]
             + [("m2", "n2", m, "L1b") for m in range(49)]
             + [("f1b2", "n1b2", m, "L2a") for m in range(20)]
             + [("f2b2", "n2b2", m, "L2b") for m in range(4)])
    assert len(plist) == 122

    pxT = np.ascontiguousarray(projected_x.T, np.float32)   # (8192,256)
    pyT = np.ascontiguousarray(projected_y.T, np.float32)

    in_maps2 = []
    meta = []  # per core: list of loss tags for its real pairs
    for k in range(NCORES):
        buf = np.zeros((NPAIR, 2, 4, 128, 256), np.float32)
        tags = []
        for t in range(NPAIR):
            pidx = k * NPAIR + t
            if pidx < len(plist):
                xg, yg, m, tag = plist[pidx]
                buf[t, 0] = groups[xg][:, m].T.reshape(4, 128, 256)
                buf[t, 1] = groups[yg][:, m].T.reshape(4, 128, 256)
                tags.append(tag)
            else:
                tags.append(None)
        gpb = np.stack([pxT[k * 1024:(k + 1) * 1024].reshape(GCHUNK, 128, 256),
                        pyT[k * 1024:(k + 1) * 1024].reshape(GCHUNK, 128, 256)], 0)
        in_maps2.append({"pairs": buf, "gp": gpb,
                         "ones128": np.ones((128, 1), np.float32)})
        meta.append(tags)

    r2 = run_bass_kernel_spmd(nc2, in_maps2, core_ids=list(range(NCORES)),
                              trace=trace)
    t2 = r2.exec_time_ns

    # ---- host epilogue: combine partial sums
    acc = {tag: {"inv": np.zeros(B, np.float64), "r": 0.0, "offd": 0.0}
           for tag in ("L1a", "L1b", "L2a", "L2b")}
    # separate x/y relu sums per tag
    racc = {tag: [0.0, 0.0] for tag in acc}
    for k in range(NCORES):
        res = r2.results[k]
        for t, tag in enumerate(meta[k]):
            if tag is None:
                continue
            acc[tag]["inv"] += res["inv_o"][t].astype(np.float64)
            sx = res["s_o"][:, t * 8:t * 8 + 4].astype(np.float64)
            sy = res["s_o"][:, t * 8 + 4:t * 8 + 8].astype(np.float64)
            gx = res["g_o"][:, t * 4:t * 4 + 2].astype(np.float64).sum()
            gy = res["g_o"][:, t * 4 + 2:t * 4 + 4].astype(np.float64).sum()
            offd_x = (gx - (sx ** 2).sum()) / (255.0 ** 2)
            offd_y = (gy - (sy ** 2).sum()) / (255.0 ** 2)
            acc[tag]["offd"] += offd_x / 2 + offd_y / 2
            racc[tag][0] += res["r_o"][:, t * 8:t * 8 + 4].astype(np.float64).sum()
            racc[tag][1] += res["r_o"][:, t * 8 + 4:t * 8 + 8].astype(np.float64).sum()

    def loss_maps(tag, M):
        a = acc[tag]
        inv = 25.0 * a["inv"] / (M * C)
        std = 25.0 * (racc[tag][0] + racc[tag][1]) / (2.0 * M * C)
        cov = 1.0 * a["offd"] / C / M
        return inv, std, cov

    inv1, std1, cov1 = loss_maps("L1a", 49)
    inv2, std2, cov2 = loss_maps("L1b", 49)
    inv3, std3, cov3 = loss_maps("L2a", 20)
    inv4, std4, cov4 = loss_maps("L2b", 4)
    local = ((inv1 + inv2) / 2 + (std1 + std2) / 2 + (cov1 + cov2) / 2
             + (inv3 + inv4) / 2 + (std3 + std4) / 2 + (cov3 + cov4) / 2)

    # global embedding loss
    Gx = np.zeros((256, 256), np.float64)
    Gy = np.zeros((256, 256), np.float64)
    ginv = np.zeros(B, np.float64)
    sx2 = sy2 = 0.0
    rgx = rgy = 0.0
    for k in range(NCORES):
        res = r2.results[k]
        gm = res["gm_o"].astype(np.float64)
        Gx += np.concatenate([gm[0], gm[1]], 0)
        Gy += np.concatenate([gm[2], gm[3]], 0)
        ginv += res["ginv_o"][0].astype(np.float64)
        sx2 += (res["gs_o"][:, 0:GCHUNK].astype(np.float64) ** 2).sum()
        sy2 += (res["gs_o"][:, GCHUNK:2 * GCHUNK].astype(np.float64) ** 2).sum()
        rgx += res["gr_o"][:, 0:GCHUNK].astype(np.float64).sum()
        rgy += res["gr_o"][:, GCHUNK:2 * GCHUNK].astype(np.float64).sum()
    inv_g = ginv / D
    std_g = rgx / D / 2 + rgy / D / 2
    offd_gx = ((Gx ** 2).sum() - sx2) / (255.0 ** 2)
    offd_gy = ((Gy ** 2).sum() - sy2) / (255.0 ** 2)
    cov_g = offd_gx / D + offd_gy / D
    glob = 25.0 * inv_g + 25.0 * std_g + 1.0 * cov_g

    out = (0.5 * glob + 0.5 * local).astype(np.float32)
    if _return_time:
        return out, (t1, t2)
    return out
